# revision 31
# baseline (speedup 1.0000x reference)
"""Trainium2 Bass kernel for nn_AdaptiveFourierFeatures.

Strategy (v2)
-------------
Math collapse (same as v1): keys are affine in f[d,f] (key_proj is
Linear(1,A)) and freq rows are identical, so attention collapses to a
per-token softmax over H*F=64 scores that are linear in x; the fourier
features contract with the MLP weights analytically (angle-addition
folds the phases into small [F,O] matrices).

Device pipeline per chunk (C=2 chunks, stacked-half layout, W=512):
    scores(2 MM) -> exp(ACT) -> den(MM) -> recip(DVE custom) -> rb(MM)
    -> at=e*rb(DVE) -> aw(2 MM) -> z=aw*trig(1 DVE op, 3D AP)
    -> pre(8 MM) -> tanh(ACT, per-chunk batched) -> wt,gs(DVE stt)
    -> out DMA (1 per chunk, fat 512B runs)

v2 changes vs v1:
  - fat-packet DMA layouts everywhere (xT plain [64,S] 1KB runs, trig
    4KB runs, out as [128,1024] scrambled layout -> 512B runs; host
    unscrambles), 2 output DMAs instead of 4
  - ones row via DVE memset (no DMA); single pk param DMA
  - PE warm-up dummies fill the input-DMA wait so real matmuls run at
    2.4GHz
  - bf16 reciprocal -> rb matmul runs bf16 (no fp32r LDWEIGHTS HIGH)
  - merged z multiply per chunk ([32,2,512] 3D AP), per-chunk batched
    tanh/wt/gs (fewer, bigger DVE/ACT ops)
  - gs = (1+tanh_g)*wt as a single DVE stt (no GpSimd hop)

Sharding: data-parallel over batch B=8, one batch element per core.
kernel(**inputs) takes FULL inputs, returns the FULL [B,S,D] output.
"""

import sys

import numpy as np
import ml_dtypes

for _p in ("/opt/trn_rl_repo", "/opt/pypackages"):
    if _p not in sys.path:
        sys.path.append(_p)

# ---- problem constants (hardcoded; kernel.py must be self-contained) ----
B, S, D, F, A, H, O = 8, 2048, 64, 16, 32, 4, 64
HD = A // H
TWO_PI = 2.0 * np.pi
N_CORES = 8
HF = H * F             # 64 score rows per token
HALF = S // 2          # 1024 tokens per half (stacked-half layout)
NCHUNKS = 2
CW = HALF // NCHUNKS   # 512 columns per chunk
NZ = 2 * F             # 32 z rows
NFEAT = D + NZ + 1     # 97 = x | z | ones

BF16 = ml_dtypes.bfloat16

# pk1 cols: [wsc 0:64 | E2q(rows 0:8) 64:192 | bias 192 | pad 193]
# pk2 cols: [O1 0:8 | O2b 8:40]   (128 rows)
# pk3:      G [97, 128]
PK1_COLS = 194

_CACHE = {}


def _make_inmaps(x: np.ndarray, params: dict) -> list:
    """Per-core input dicts for run_bass_kernel_spmd (shared w/ test.py)."""
    in_maps = []
    for b in range(N_CORES):
        m = dict(params)
        m["xT"] = np.ascontiguousarray(x[b].T).astype(BF16)
        in_maps.append(m)
    return in_maps


def _finish(x: np.ndarray, res) -> np.ndarray:
    """Host-side unscramble + residual: out = x + 0.25 * gated4."""
    outs = []
    for b in range(N_CORES):
        g = np.asarray(res.results[b]["out"]).astype(np.float32)
        # [128, 1024] -> [p, c, h, j, d]; token = 1024c + 512h + 128j + p
        g = g.reshape(128, NCHUNKS, 2, 4, D).transpose(1, 2, 3, 0, 4)
        outs.append(g.reshape(S, D))
    gs = np.stack(outs, axis=0)
    return (x + 0.25 * gs).astype(np.float32)


def _build_program(ndum: int = 5, use_bias: bool = False,
                   light_exit: bool = True):
    """Build the 8-core SPMD bass program (per-core shapes).

    Chunk c covers the contiguous token block [c*1024, (c+1)*1024); its
    stacked halves are the first/second 512 tokens of that block, so every
    DMA moves long contiguous per-partition runs.
    """
    import concourse.bass as bass
    import concourse.bacc as bacc
    import concourse.tile as tile
    from concourse import mybir

    class _LightExitTileContext(tile.TileContext):
        """Single-TileContext program: keep the exit drain (waits out all
        DMA-completion sems), one barrier, and the DMA-queue/semaphore
        clear, but skip the second all-engine barrier — the NEFF postamble
        provides the final synchronization, and the clears it races against
        are themselves resets."""

        def _drain_and_barrier(self, tick_clock, wait_clock):
            from concourse.vector_clock import ScopedClock
            drain_inst = self.nc.sync.drain()
            wait_clock.add_sem_waits(
                drain_inst.ins, ScopedClock({None: tick_clock.global_clock})
            )
            self.nc.all_engine_barrier()
            popped = self.nc._tile_sem_poison_stack.pop()
            assert popped is self._sem_poison
            self.nc.clear_and_free_semaphores(
                list(self.sems.allocated().values()))

    dt = mybir.dt
    AF = mybir.ActivationFunctionType
    ALU = mybir.AluOpType

    nc = bacc.Bacc("TRN2", target_bir_lowering=False, debug=False,
                   enable_asserts=True, num_devices=N_CORES,
                   enable_partition_id=True)

    BLK = S // NCHUNKS  # 1024 tokens per chunk block

    xT = nc.dram_tensor("xT", [D, S], dt.bfloat16, kind="ExternalInput").ap()
    trig = nc.dram_tensor("trig", [NZ, S], dt.bfloat16, kind="ExternalInput").ap()
    pk1 = nc.dram_tensor("pk1", [64, PK1_COLS], dt.bfloat16, kind="ExternalInput").ap()
    pk2 = nc.dram_tensor("pk2", [128, 41], dt.bfloat16, kind="ExternalInput").ap()
    pk3 = nc.dram_tensor("pk3", [NFEAT, 128], dt.bfloat16, kind="ExternalInput").ap()
    ones1 = nc.dram_tensor("ones1", [1, S], dt.bfloat16, kind="ExternalInput").ap()
    # device returns gated*4 in bf16, scrambled [128, 1024]; host finishes
    out_d = nc.dram_tensor("out", [128, 2 * NCHUNKS * 4 * D], dt.bfloat16,
                           kind="ExternalOutput").ap()

    tcls = _LightExitTileContext if light_exit else tile.TileContext
    with tcls(nc) as tc:
        with (
            tc.tile_pool(name="const", bufs=1) as cpool,
            tc.tile_pool(name="sb", bufs=1) as sb,
            tc.tile_pool(name="we1", bufs=2) as we1,
            tc.tile_pool(name="wk", bufs=2) as wk,
            tc.tile_pool(name="ps", bufs=4, space="PSUM") as ps,
        ):
            # ---- DMAs first: they gate everything ----
            # CZ = [x^T (0:64) | z (64:96) | ones (96)], plain token order
            cz = sb.tile([NFEAT, S], dt.bfloat16)
            for c in range(NCHUNKS):
                nc.sync.dma_start(out=cz[0:D, c * BLK:(c + 1) * BLK],
                                  in_=xT[:, c * BLK:(c + 1) * BLK])
            # ones row: one 4KB packet
            nc.sync.dma_start(out=cz[NFEAT - 1:NFEAT, :], in_=ones1[:])
            # trig rows on partitions 64..95 (lane-aligned with cz z rows)
            c_trig = cpool.tile([96, S], dt.bfloat16)
            nc.sync.dma_start(out=c_trig[64:96, :], in_=trig[:])

            # params on the scalar(ACT) ring, ahead of the ACT table load
            c_pk1 = cpool.tile([64, PK1_COLS], dt.bfloat16)
            nc.scalar.dma_start(out=c_pk1[:], in_=pk1[:])
            c_pk2 = cpool.tile([128, 41], dt.bfloat16)
            nc.scalar.dma_start(out=c_pk2[:], in_=pk2[:])
            c_g = cpool.tile([NFEAT, 128], dt.bfloat16)
            nc.scalar.dma_start(out=c_g[:], in_=pk3[:])

            c_wsc = c_pk1[0:D, 0:64]
            c_e2 = c_pk1[0:8, 64:192]
            c_o1 = c_pk2[0:128, 0:8]
            c_o2 = c_pk2[0:128, 8:40]

            c_bsc = None
            if use_bias:
                c_bsc = cpool.tile([128, 1], dt.float32)
                nc.vector.tensor_copy(c_bsc[:], c_pk2[:, 40:41])

            # warm up the activation table set (exp/tanh share one set)
            warm = cpool.tile([1, 2], dt.float32)
            nc.vector.memset(warm[:], 0.0)
            nc.scalar.activation(warm[:], warm[:], AF.Exp)

            # (no PE warm-up dummies: HAM never un-throttles in this
            # environment — measured cold-formula matmul durations kernel-
            # wide even under dense dummy streams — so dummies only risk
            # delaying the first scores matmul)

            from concourse.dve_ops import (
                RECIP_APPROX_FAST_CONSTS as _RC,
                RECIPROCAL_APPROX_FAST as _RAF,
            )

            # stage-major emission across chunks so engine FIFOs match the
            # data-ready order
            ch = [dict(lo=c * BLK) for c in range(NCHUNKS)]

            # -- scores: s2[h*64+hf, q] = sum_d x^T[d, tok] wsc[d, hf]
            for st in ch:
                s2 = ps.tile([128, 2 * CW], dt.float32, tag="big")
                st["s2"] = s2
                for h in range(2):
                    t0 = st["lo"] + h * CW
                    nc.tensor.matmul(s2[h * 64:(h + 1) * 64, 0:CW], c_wsc,
                                     cz[0:D, t0:t0 + CW],
                                     tile_position=(0, h * 64))
            # -- exp (bias adds the alpha-offset term when nonzero)
            for st in ch:
                e1 = we1.tile([128, CW], dt.bfloat16, tag="e1")
                st["e1"] = e1
                if use_bias:
                    nc.scalar.activation(e1[:], st["s2"][:, 0:CW], AF.Exp,
                                         bias=c_bsc[:])
                else:
                    nc.scalar.activation(e1[:], st["s2"][:, 0:CW], AF.Exp)
            # -- denominators, written into the retiring s2 slot (rows 0:8)
            for st in ch:
                den = st["s2"][0:8, 0:CW]
                st["den"] = den
                nc.tensor.matmul(den, c_o1, st["e1"][:], tile_position=(0, 0))
            # -- reciprocal (fast Newton approx), bf16 out for a bf16 rb MM
            for st in ch:
                rec = we1.tile([8, CW], dt.bfloat16, tag="rec")
                st["rec"] = rec
                nc.vector._custom_dve(_RAF, out=rec[:], in0=st["den"],
                                      s0=_RC["s0"], s1=_RC["s1"],
                                      imm2=_RC["imm2"])
            # -- broadcast 1/den to all 128 rows (0.25 head-mean in E2q),
            #    overwriting the s2 slot in place
            for st in ch:
                rb = st["s2"][:, 0:CW]
                st["rb"] = rb
                nc.tensor.matmul(rb, c_e2, st["rec"][:], tile_position=(0, 0))
            # -- attn/4 = e1 * rb   (DVE: GpSimd cannot read PSUM)
            for st in ch:
                at = we1.tile([128, CW], dt.bfloat16, tag="at")
                st["at"] = at
                nc.vector.tensor_mul(at[:], st["e1"][:], st["rb"])
            # -- aw rows (sin/cos duplicated) for both halves into one
            #    [96, 2*CW] psum tile: h0 -> cols 0:CW, h1 -> cols CW:2CW
            for st in ch:
                aw = ps.tile([96, 2 * CW], dt.float32, tag="big")
                st["aw"] = aw
                for h in range(2):
                    nc.tensor.matmul(
                        aw[64:96, h * CW:(h + 1) * CW],
                        c_o2[h * 64:(h + 1) * 64, :],
                        st["at"][h * 64:(h + 1) * 64, :],
                        tile_position=(h * 64, 64),
                    )
            # -- z = aw * trig into CZ rows 64:96, per half so chunk 0's
            #    pre matmuls can start as soon as its h0 z lands
            for c, st in enumerate(ch):
                for h in range(2):
                    t0 = st["lo"] + h * CW
                    nc.vector.tensor_mul(
                        cz[64:96, t0:t0 + CW],
                        st["aw"][64:96, h * CW:(h + 1) * CW],
                        c_trig[64:96, t0:t0 + CW])

            # -- per-chunk tail: pre MM -> tanh_g/tanh_p -> wt -> gs -> DMA
            # pre cols: h*CW + j*128 ; out cols: (c*8 + h*4 + j)*64 + d
            for c, st in enumerate(ch):
                pre = ps.tile([128, 2 * CW], dt.float32, tag="big")
                for h in range(2):
                    for j in range(CW // 128):
                        t0 = st["lo"] + h * CW + j * 128
                        nc.tensor.matmul(
                            pre[:, h * CW + j * 128:h * CW + (j + 1) * 128],
                            cz[:, t0:t0 + 128], c_g,
                            tile_position=(0, 0),
                        )
                pre_v = pre[:].rearrange("p (m o) -> p m o", o=128)
                thg = wk.tile([128, CW], dt.bfloat16, tag="thg")
                thp = wk.tile([128, CW], dt.bfloat16, tag="thp")
                thg_v = thg[:].rearrange("p (m o) -> p m o", o=64)
                thp_v = thp[:].rearrange("p (m o) -> p m o", o=64)
                # tanh(pre/2); sigmoid(a)=0.5+0.5*tanh(a/2).  tanh_p first,
                # then wt, THEN tanh_g: wt only needs tanh_p, and this
                # emission order keeps the scheduler from chaining wt
                # behind tanh_g (the scheduler serializes per emission
                # order, so wt overlaps tanh_g this way)
                nc.scalar.activation(thp_v, pre_v[:, :, 64:128], AF.Tanh,
                                     scale=0.5)
                # wt = (1+tanh_p) * pre_p   [2*silu]
                wt = wk.tile([128, CW], dt.bfloat16, tag="wt")
                wt_v = wt[:].rearrange("p (m o) -> p m o", o=64)
                nc.vector.scalar_tensor_tensor(
                    wt_v, thp_v, 1.0, pre_v[:, :, 64:128], ALU.add, ALU.mult)
                nc.scalar.activation(thg_v, pre_v[:, :, 0:64], AF.Tanh,
                                     scale=0.5)
                # gated*4 = (1+tanh_g) * wt   (single stt beats a 2x
                # mul+add pair here: 685 vs 2x~423 measured)
                gs = wk.tile([128, CW], dt.bfloat16, tag="gs")
                nc.vector.scalar_tensor_tensor(
                    gs[:], thg[:], 1.0, wt[:], ALU.add, ALU.mult)
                # one output DMA per chunk (dst cols c*512:(c+1)*512); the
                # two chunks ride different rings so the transfers overlap
                eng = nc.scalar if c == 0 else nc.sync
                eng.dma_start(out=out_d[:, c * 512:(c + 1) * 512], in_=gs[:])

    nc.compile()
    return nc


def _fold_params(inputs):
    """Host-side constant folding (float64).  Returns per-core arrays."""
    f = (np.asarray(inputs["freq_matrix"], np.float64)
         * np.asarray(inputs["freq_scale"], np.float64))
    g = f[0]
    gc = 0.5 * (g.max() + g.min())
    gsh = g - gc

    Wq = np.asarray(inputs["Wq"], np.float64)
    bq = np.asarray(inputs["bq"], np.float64)
    Wk1 = np.asarray(inputs["Wk1"], np.float64)
    Wqi = np.asarray(inputs["Wqi"], np.float64)
    bqi = np.asarray(inputs["bqi"], np.float64)
    Wki = np.asarray(inputs["Wki"], np.float64)
    ph = np.asarray(inputs["phase"], np.float64)

    u = Wki @ Wk1[:, 0]
    Wqq = Wqi @ Wq
    bqq = Wqi @ bq + bqi
    u_h = u.reshape(H, HD)
    M_alpha = np.einsum("he,hed->hd", u_h, Wqq.reshape(H, HD, D)) / np.sqrt(HD)
    c_alpha = np.einsum("he,he->h", u_h, bqq.reshape(H, HD)) / np.sqrt(HD)

    W_score = np.einsum("hd,f->dhf", M_alpha, gsh).reshape(D, HF)
    b_score = np.einsum("h,f->hf", c_alpha, gsh).reshape(HF)
    b_score2 = np.concatenate([b_score, b_score])  # [128]

    t = np.linspace(0.0, 1.0, S)
    theta = TWO_PI * t[:, None] * g[None, :]
    trig = np.concatenate([np.sin(theta).T, np.cos(theta).T], 0)  # [2F, S]

    cph, sph = np.cos(ph), np.sin(ph)

    def fold_mlp(W):
        W = np.asarray(W, np.float64)
        Wx = W[:, :D]
        Wf = W[:, D:].reshape(O, D, 2 * F)
        Ws, Wc = Wf[:, :, :F], Wf[:, :, F:]
        Us = np.einsum("df,odf->fo", cph, Ws) - np.einsum("df,odf->fo", sph, Wc)
        Uc = np.einsum("df,odf->fo", sph, Ws) + np.einsum("df,odf->fo", cph, Wc)
        return Wx, Us, Uc

    Wgx, Ugs, Ugc = fold_mlp(inputs["Wg"])
    Wpx, Ups, Upc = fold_mlp(inputs["Wp"])
    bg = np.asarray(inputs["bg"], np.float64)
    bp = np.asarray(inputs["bp"], np.float64)

    # G rows: 0:64 x | 64:80 z_sin | 80:96 z_cos | 96 ones(bias)
    G = np.zeros((NFEAT, 128))
    G[0:D, 0:64] = Wgx.T
    G[D:D + F, 0:64] = Ugs
    G[D + F:D + 2 * F, 0:64] = Ugc
    G[NFEAT - 1, 0:64] = bg
    G[0:D, 64:128] = Wpx.T
    G[D:D + F, 64:128] = Ups
    G[D + F:D + 2 * F, 64:128] = Upc
    G[NFEAT - 1, 64:128] = bp

    # indicator matrices for the softmax plumbing
    p = np.arange(128)
    O1 = (p[:, None] // 16 == np.arange(8)[None, :]).astype(np.float64)
    E2q = 0.25 * (np.arange(8)[:, None] == p[None, :] // 16).astype(np.float64)
    O2 = ((p[:, None] % 16) == (np.arange(32)[None, :] % 16)).astype(np.float64)

    pk1 = np.zeros((64, PK1_COLS))
    pk1[0:D, 0:64] = W_score
    pk1[0:8, 64:192] = E2q
    pk2 = np.zeros((128, 41))
    pk2[:, 0:8] = O1
    pk2[:, 8:40] = O2
    pk2[:, 40] = b_score2

    return dict(
        trig=trig.astype(BF16),
        pk1=pk1.astype(BF16),
        pk2=pk2.astype(BF16),
        pk3=G.astype(BF16),
        ones1=np.ones((1, S), BF16),
    ), gsh, M_alpha, c_alpha


def _numpy_fallback(inputs):
    """Exact collapsed computation in numpy (general freq rows)."""
    x = np.asarray(inputs["x"], np.float64)
    f = (np.asarray(inputs["freq_matrix"], np.float64)
         * np.asarray(inputs["freq_scale"], np.float64))
    Wq = np.asarray(inputs["Wq"], np.float64); bq = np.asarray(inputs["bq"], np.float64)
    Wk1 = np.asarray(inputs["Wk1"], np.float64); bk1 = np.asarray(inputs["bk1"], np.float64)
    Wqi = np.asarray(inputs["Wqi"], np.float64); bqi = np.asarray(inputs["bqi"], np.float64)
    Wki = np.asarray(inputs["Wki"], np.float64); bki = np.asarray(inputs["bki"], np.float64)
    Wg = np.asarray(inputs["Wg"], np.float64); bg = np.asarray(inputs["bg"], np.float64)
    Wp = np.asarray(inputs["Wp"], np.float64); bp = np.asarray(inputs["bp"], np.float64)
    ph = np.asarray(inputs["phase"], np.float64)

    u = Wki @ Wk1[:, 0]
    v = Wki @ bk1 + bki
    q = (x @ Wq.T + bq) @ Wqi.T + bqi
    qh = q.reshape(B, S, H, HD)
    alpha = np.einsum("bshe,he->bsh", qh, u.reshape(H, HD)) / np.sqrt(HD)
    beta = np.einsum("bshe,he->bsh", qh, v.reshape(H, HD)) / np.sqrt(HD)
    sc = alpha[..., None, :, None] * f[None, None, :, None, :] \
        + beta[..., None, :, None]
    sc -= sc.max(-1, keepdims=True)
    e = np.exp(sc)
    attn = e / e.sum(-1, keepdims=True)
    aw = attn.mean(-2)
    t = np.linspace(0.0, 1.0, S)
    sig = TWO_PI * t[None, :, None, None] * f[None, None] + ph[None, None]
    ffs = np.sin(sig) * aw
    ffc = np.cos(sig) * aw
    ff = np.concatenate([ffs, ffc], axis=-1).reshape(B, S, D * 2 * F)
    ci = np.concatenate([x, ff], axis=-1)
    gate = 1.0 / (1.0 + np.exp(-(ci @ Wg.T + bg)))
    pp = ci @ Wp.T + bp
    silu = pp / (1.0 + np.exp(-pp))
    return (x + gate * silu).astype(np.float32)


def kernel(**inputs) -> np.ndarray:
    x = np.asarray(inputs["x"], np.float32)

    f = (np.asarray(inputs["freq_matrix"], np.float64)
         * np.asarray(inputs["freq_scale"], np.float64))
    if not np.all(f == f[0:1]):
        return _numpy_fallback(inputs)

    params, gsh, M_alpha, c_alpha = _fold_params(inputs)

    # exp-overflow guard (score = alpha*(g-gc); needs |score| < ~85)
    xmaxn = np.linalg.norm(x.reshape(-1, D), axis=1).max()
    amax = np.linalg.norm(M_alpha, axis=1).max() * xmaxn + np.abs(c_alpha).max()
    if amax * np.abs(gsh).max() > 85.0:
        return _numpy_fallback(inputs)

    use_bias = bool(np.abs(np.asarray(params["pk2"], np.float64)[:, 40]).max() > 0)
    key = f"prog{int(use_bias)}"
    if key not in _CACHE:
        _CACHE[key] = _build_program(use_bias=use_bias)
    nc = _CACHE[key]

    from concourse.bass_utils import run_bass_kernel_spmd

    in_maps = _make_inmaps(x, params)
    res = run_bass_kernel_spmd(nc, in_maps, core_ids=list(range(N_CORES)))
    return _finish(x, res)


if __name__ == "__main__":
    import reference
    ins = {k: np.asarray(v) for k, v in reference.setup_inputs().items()}
    got = kernel(**ins)
    import jax.numpy as jnp
    exp = np.asarray(reference.reference(**{k: jnp.asarray(v) for k, v in ins.items()}))
    err = np.linalg.norm(got - exp) / np.linalg.norm(exp)
    print("rel err:", err)


# revision 33
# speedup vs baseline: 1.0372x; 1.0372x over previous
"""Trainium2 Bass kernel for nn_AdaptiveFourierFeatures.

Strategy (v2)
-------------
Math collapse (same as v1): keys are affine in f[d,f] (key_proj is
Linear(1,A)) and freq rows are identical, so attention collapses to a
per-token softmax over H*F=64 scores that are linear in x; the fourier
features contract with the MLP weights analytically (angle-addition
folds the phases into small [F,O] matrices).

Device pipeline per chunk (C=2 chunks, stacked-half layout, W=512):
    scores(2 MM) -> exp(ACT) -> den(MM) -> recip(DVE custom) -> rb(MM)
    -> at=e*rb(DVE) -> aw(2 MM) -> z=aw*trig(1 DVE op, 3D AP)
    -> pre(8 MM) -> tanh(ACT, per-chunk batched) -> wt,gs(DVE stt)
    -> out DMA (1 per chunk, fat 512B runs)

v2 changes vs v1:
  - fat-packet DMA layouts everywhere (xT plain [64,S] 1KB runs, trig
    4KB runs, out as [128,1024] scrambled layout -> 512B runs; host
    unscrambles), 2 output DMAs instead of 4
  - ones row via DVE memset (no DMA); single pk param DMA
  - PE warm-up dummies fill the input-DMA wait so real matmuls run at
    2.4GHz
  - bf16 reciprocal -> rb matmul runs bf16 (no fp32r LDWEIGHTS HIGH)
  - merged z multiply per chunk ([32,2,512] 3D AP), per-chunk batched
    tanh/wt/gs (fewer, bigger DVE/ACT ops)
  - gs = (1+tanh_g)*wt as a single DVE stt (no GpSimd hop)

Sharding: data-parallel over batch B=8, one batch element per core.
kernel(**inputs) takes FULL inputs, returns the FULL [B,S,D] output.
"""

import sys

import numpy as np
import ml_dtypes

for _p in ("/opt/trn_rl_repo", "/opt/pypackages"):
    if _p not in sys.path:
        sys.path.append(_p)

# ---- problem constants (hardcoded; kernel.py must be self-contained) ----
B, S, D, F, A, H, O = 8, 2048, 64, 16, 32, 4, 64
HD = A // H
TWO_PI = 2.0 * np.pi
N_CORES = 8
HF = H * F             # 64 score rows per token
HALF = S // 2          # 1024 tokens per half (stacked-half layout)
NCHUNKS = 2
CW = HALF // NCHUNKS   # 512 columns per chunk
NZ = 2 * F             # 32 z rows
NFEAT = D + NZ + 1     # 97 = x | z | ones

BF16 = ml_dtypes.bfloat16

# pk1 cols: [wsc 0:64 | E2q(rows 0:8) 64:192 | bias 192 | pad 193]
# pk2 cols: [O1 0:8 | O2b 8:40]   (128 rows)
# pk3:      G [97, 128]
PK1_COLS = 194

_CACHE = {}


def _make_inmaps(x: np.ndarray, params: dict) -> list:
    """Per-core input dicts for run_bass_kernel_spmd (shared w/ test.py)."""
    in_maps = []
    for b in range(N_CORES):
        m = dict(params)
        m["xT"] = np.ascontiguousarray(x[b].T).astype(BF16)
        in_maps.append(m)
    return in_maps


def _finish(x: np.ndarray, res) -> np.ndarray:
    """Host-side unscramble + residual: out = x + 0.25 * gated4."""
    outs = []
    for b in range(N_CORES):
        g = np.asarray(res.results[b]["out"]).astype(np.float32)
        # [128, 1024] -> [p, c, h, j, d]; token = 1024c + 512h + 128j + p
        g = g.reshape(128, NCHUNKS, 2, 4, D).transpose(1, 2, 3, 0, 4)
        outs.append(g.reshape(S, D))
    gs = np.stack(outs, axis=0)
    return (x + 0.25 * gs).astype(np.float32)


def _build_program(ndum: int = 5, use_bias: bool = False,
                   light_exit: bool = True):
    """Build the 8-core SPMD bass program (per-core shapes).

    Chunk c covers the contiguous token block [c*1024, (c+1)*1024); its
    stacked halves are the first/second 512 tokens of that block, so every
    DMA moves long contiguous per-partition runs.
    """
    import concourse.bass as bass
    import concourse.bacc as bacc
    import concourse.tile as tile
    from concourse import mybir

    class _LightExitTileContext(tile.TileContext):
        """Single-TileContext program: keep the exit drain (waits out all
        DMA-completion sems), one barrier, and the DMA-queue/semaphore
        clear, but skip the second all-engine barrier — the NEFF postamble
        provides the final synchronization, and the clears it races against
        are themselves resets."""

        def _drain_and_barrier(self, tick_clock, wait_clock):
            from concourse.vector_clock import ScopedClock
            drain_inst = self.nc.sync.drain()
            wait_clock.add_sem_waits(
                drain_inst.ins, ScopedClock({None: tick_clock.global_clock})
            )
            self.nc.all_engine_barrier()
            popped = self.nc._tile_sem_poison_stack.pop()
            assert popped is self._sem_poison
            self.nc.clear_and_free_semaphores(
                list(self.sems.allocated().values()))

    dt = mybir.dt
    AF = mybir.ActivationFunctionType
    ALU = mybir.AluOpType

    nc = bacc.Bacc("TRN2", target_bir_lowering=False, debug=False,
                   enable_asserts=True, num_devices=N_CORES,
                   enable_partition_id=True)

    BLK = S // NCHUNKS  # 1024 tokens per chunk block

    xT = nc.dram_tensor("xT", [D, S], dt.bfloat16, kind="ExternalInput").ap()
    trig = nc.dram_tensor("trig", [NZ, S], dt.bfloat16, kind="ExternalInput").ap()
    pk1 = nc.dram_tensor("pk1", [64, PK1_COLS], dt.bfloat16, kind="ExternalInput").ap()
    pk2 = nc.dram_tensor("pk2", [128, 41], dt.bfloat16, kind="ExternalInput").ap()
    pk3 = nc.dram_tensor("pk3", [NFEAT, 128], dt.bfloat16, kind="ExternalInput").ap()
    ones1 = nc.dram_tensor("ones1", [1, S], dt.bfloat16, kind="ExternalInput").ap()
    # device returns gated*4 in bf16, scrambled [128, 1024]; host finishes
    out_d = nc.dram_tensor("out", [128, 2 * NCHUNKS * 4 * D], dt.bfloat16,
                           kind="ExternalOutput").ap()

    tcls = _LightExitTileContext if light_exit else tile.TileContext
    with tcls(nc) as tc:
        with (
            tc.tile_pool(name="const", bufs=1) as cpool,
            tc.tile_pool(name="sb", bufs=1) as sb,
            tc.tile_pool(name="we1", bufs=2) as we1,
            tc.tile_pool(name="wk", bufs=2) as wk,
            tc.tile_pool(name="ps", bufs=4, space="PSUM") as ps,
        ):
            # ---- DMAs first: they gate everything ----
            # CZ = [x^T (0:64) | z (64:96) | ones (96)], plain token order
            cz = sb.tile([NFEAT, S], dt.bfloat16)
            for c in range(NCHUNKS):
                nc.sync.dma_start(out=cz[0:D, c * BLK:(c + 1) * BLK],
                                  in_=xT[:, c * BLK:(c + 1) * BLK])
            # ones row: one 4KB packet
            nc.sync.dma_start(out=cz[NFEAT - 1:NFEAT, :], in_=ones1[:])
            # trig rows on partitions 64..95 (lane-aligned with cz z rows)
            c_trig = cpool.tile([96, S], dt.bfloat16)
            nc.sync.dma_start(out=c_trig[64:96, :], in_=trig[:])

            # params on the scalar(ACT) ring, ahead of the ACT table load
            c_pk1 = cpool.tile([64, PK1_COLS], dt.bfloat16)
            nc.scalar.dma_start(out=c_pk1[:], in_=pk1[:])
            c_pk2 = cpool.tile([128, 41], dt.bfloat16)
            nc.scalar.dma_start(out=c_pk2[:], in_=pk2[:])
            c_g = cpool.tile([NFEAT, 128], dt.bfloat16)
            nc.scalar.dma_start(out=c_g[:], in_=pk3[:])

            c_wsc = c_pk1[0:D, 0:64]
            c_e2 = c_pk1[0:8, 64:192]
            c_o1 = c_pk2[0:128, 0:8]
            c_o2 = c_pk2[0:128, 8:40]

            c_bsc = None
            if use_bias:
                c_bsc = cpool.tile([128, 1], dt.float32)
                nc.vector.tensor_copy(c_bsc[:], c_pk2[:, 40:41])

            # warm up the activation table set (exp/tanh share one set)
            warm = cpool.tile([1, 2], dt.float32)
            nc.vector.memset(warm[:], 0.0)
            nc.scalar.activation(warm[:], warm[:], AF.Exp)

            # (no PE warm-up dummies: HAM never un-throttles in this
            # environment — cold-formula matmul durations measured kernel-
            # wide even under dense dummy streams — so dummies only risk
            # delaying the first scores matmul)

            from concourse.dve_ops import (
                RECIP_APPROX_FAST_CONSTS as _RC,
                RECIPROCAL_APPROX_FAST as _RAF,
            )

            # stage-major emission across chunks so engine FIFOs match the
            # data-ready order
            ch = [dict(lo=c * BLK) for c in range(NCHUNKS)]

            # -- scores: s2[h*64+hf, q] = sum_d x^T[d, tok] wsc[d, hf]
            for st in ch:
                s2 = ps.tile([128, 2 * CW], dt.float32, tag="big")
                st["s2"] = s2
                for h in range(2):
                    t0 = st["lo"] + h * CW
                    nc.tensor.matmul(s2[h * 64:(h + 1) * 64, 0:CW], c_wsc,
                                     cz[0:D, t0:t0 + CW],
                                     tile_position=(0, h * 64))
            # -- exp (bias adds the alpha-offset term when nonzero)
            for st in ch:
                e1 = we1.tile([128, CW], dt.bfloat16, tag="e1")
                st["e1"] = e1
                if use_bias:
                    nc.scalar.activation(e1[:], st["s2"][:, 0:CW], AF.Exp,
                                         bias=c_bsc[:])
                else:
                    nc.scalar.activation(e1[:], st["s2"][:, 0:CW], AF.Exp)
            # -- denominators, written into the retiring s2 slot (rows 0:8)
            for st in ch:
                den = st["s2"][0:8, 0:CW]
                st["den"] = den
                nc.tensor.matmul(den, c_o1, st["e1"][:], tile_position=(0, 0))
            # -- reciprocal (fast Newton approx), bf16 out for a bf16 rb MM
            for st in ch:
                rec = we1.tile([8, CW], dt.bfloat16, tag="rec")
                st["rec"] = rec
                nc.vector._custom_dve(_RAF, out=rec[:], in0=st["den"],
                                      s0=_RC["s0"], s1=_RC["s1"],
                                      imm2=_RC["imm2"])
            # -- broadcast 1/den to all 128 rows (0.25 head-mean in E2q),
            #    overwriting the s2 slot in place
            for st in ch:
                rb = st["s2"][:, 0:CW]
                st["rb"] = rb
                nc.tensor.matmul(rb, c_e2, st["rec"][:], tile_position=(0, 0))
            # -- attn/4 = e1 * rb   (DVE: GpSimd cannot read PSUM)
            for st in ch:
                at = we1.tile([128, CW], dt.bfloat16, tag="at")
                st["at"] = at
                nc.vector.tensor_mul(at[:], st["e1"][:], st["rb"])
            # -- aw rows (sin/cos duplicated) for both halves into one
            #    [96, 2*CW] psum tile: h0 -> cols 0:CW, h1 -> cols CW:2CW
            for st in ch:
                aw = ps.tile([96, 2 * CW], dt.float32, tag="big")
                st["aw"] = aw
                for h in range(2):
                    nc.tensor.matmul(
                        aw[64:96, h * CW:(h + 1) * CW],
                        c_o2[h * 64:(h + 1) * 64, :],
                        st["at"][h * 64:(h + 1) * 64, :],
                        tile_position=(h * 64, 64),
                    )
            # -- z = aw * trig into CZ rows 64:96, per half so chunk 0's
            #    pre matmuls can start as soon as its h0 z lands
            for c, st in enumerate(ch):
                for h in range(2):
                    t0 = st["lo"] + h * CW
                    nc.vector.tensor_mul(
                        cz[64:96, t0:t0 + CW],
                        st["aw"][64:96, h * CW:(h + 1) * CW],
                        c_trig[64:96, t0:t0 + CW])

            # -- per-chunk tail: pre MM -> tanh_g/tanh_p -> wt -> gs -> DMA
            # pre cols: h*CW + j*128 ; out cols: (c*8 + h*4 + j)*64 + d
            for c, st in enumerate(ch):
                pre = ps.tile([128, 2 * CW], dt.float32, tag="big")
                for h in range(2):
                    for j in range(CW // 128):
                        t0 = st["lo"] + h * CW + j * 128
                        nc.tensor.matmul(
                            pre[:, h * CW + j * 128:h * CW + (j + 1) * 128],
                            cz[:, t0:t0 + 128], c_g,
                            tile_position=(0, 0),
                        )
                pre_v = pre[:].rearrange("p (m o) -> p m o", o=128)
                thg = wk.tile([128, CW], dt.bfloat16, tag="thg")
                thp = wk.tile([128, CW], dt.bfloat16, tag="thp")
                thg_v = thg[:].rearrange("p (m o) -> p m o", o=64)
                thp_v = thp[:].rearrange("p (m o) -> p m o", o=64)
                # tanh(pre/2); sigmoid(a)=0.5+0.5*tanh(a/2).  tanh_p first,
                # then wt, THEN tanh_g: wt only needs tanh_p, and this
                # emission order keeps the scheduler from chaining wt
                # behind tanh_g (the scheduler serializes per emission
                # order, so wt overlaps tanh_g this way)
                nc.scalar.activation(thp_v, pre_v[:, :, 64:128], AF.Tanh,
                                     scale=0.5)
                # wt = (1+tanh_p) * pre_p   [2*silu]
                wt = wk.tile([128, CW], dt.bfloat16, tag="wt")
                wt_v = wt[:].rearrange("p (m o) -> p m o", o=64)
                nc.vector.scalar_tensor_tensor(
                    wt_v, thp_v, 1.0, pre_v[:, :, 64:128], ALU.add, ALU.mult)
                nc.scalar.activation(thg_v, pre_v[:, :, 0:64], AF.Tanh,
                                     scale=0.5)
                # gated*4 = (1+tanh_g) * wt
                gs = wk.tile([128, CW], dt.bfloat16, tag="gs")
                nc.vector.scalar_tensor_tensor(
                    gs[:], thg[:], 1.0, wt[:], ALU.add, ALU.mult)
                # one output DMA per chunk (dst cols c*512:(c+1)*512); the
                # two chunks ride different rings so the transfers overlap
                eng = nc.scalar if c == 0 else nc.sync
                eng.dma_start(out=out_d[:, c * 512:(c + 1) * 512], in_=gs[:])

    nc.compile()
    return nc


def _fold_params(inputs):
    """Host-side constant folding (float64).  Returns per-core arrays."""
    f = (np.asarray(inputs["freq_matrix"], np.float64)
         * np.asarray(inputs["freq_scale"], np.float64))
    g = f[0]
    gc = 0.5 * (g.max() + g.min())
    gsh = g - gc

    Wq = np.asarray(inputs["Wq"], np.float64)
    bq = np.asarray(inputs["bq"], np.float64)
    Wk1 = np.asarray(inputs["Wk1"], np.float64)
    Wqi = np.asarray(inputs["Wqi"], np.float64)
    bqi = np.asarray(inputs["bqi"], np.float64)
    Wki = np.asarray(inputs["Wki"], np.float64)
    ph = np.asarray(inputs["phase"], np.float64)

    u = Wki @ Wk1[:, 0]
    Wqq = Wqi @ Wq
    bqq = Wqi @ bq + bqi
    u_h = u.reshape(H, HD)
    M_alpha = np.einsum("he,hed->hd", u_h, Wqq.reshape(H, HD, D)) / np.sqrt(HD)
    c_alpha = np.einsum("he,he->h", u_h, bqq.reshape(H, HD)) / np.sqrt(HD)

    W_score = np.einsum("hd,f->dhf", M_alpha, gsh).reshape(D, HF)
    b_score = np.einsum("h,f->hf", c_alpha, gsh).reshape(HF)
    b_score2 = np.concatenate([b_score, b_score])  # [128]

    t = np.linspace(0.0, 1.0, S)
    theta = TWO_PI * t[:, None] * g[None, :]
    trig = np.concatenate([np.sin(theta).T, np.cos(theta).T], 0)  # [2F, S]

    cph, sph = np.cos(ph), np.sin(ph)

    def fold_mlp(W):
        W = np.asarray(W, np.float64)
        Wx = W[:, :D]
        Wf = W[:, D:].reshape(O, D, 2 * F)
        Ws, Wc = Wf[:, :, :F], Wf[:, :, F:]
        Us = np.einsum("df,odf->fo", cph, Ws) - np.einsum("df,odf->fo", sph, Wc)
        Uc = np.einsum("df,odf->fo", sph, Ws) + np.einsum("df,odf->fo", cph, Wc)
        return Wx, Us, Uc

    Wgx, Ugs, Ugc = fold_mlp(inputs["Wg"])
    Wpx, Ups, Upc = fold_mlp(inputs["Wp"])
    bg = np.asarray(inputs["bg"], np.float64)
    bp = np.asarray(inputs["bp"], np.float64)

    # G rows: 0:64 x | 64:80 z_sin | 80:96 z_cos | 96 ones(bias)
    G = np.zeros((NFEAT, 128))
    G[0:D, 0:64] = Wgx.T
    G[D:D + F, 0:64] = Ugs
    G[D + F:D + 2 * F, 0:64] = Ugc
    G[NFEAT - 1, 0:64] = bg
    G[0:D, 64:128] = Wpx.T
    G[D:D + F, 64:128] = Ups
    G[D + F:D + 2 * F, 64:128] = Upc
    G[NFEAT - 1, 64:128] = bp

    # indicator matrices for the softmax plumbing
    p = np.arange(128)
    O1 = (p[:, None] // 16 == np.arange(8)[None, :]).astype(np.float64)
    E2q = 0.25 * (np.arange(8)[:, None] == p[None, :] // 16).astype(np.float64)
    O2 = ((p[:, None] % 16) == (np.arange(32)[None, :] % 16)).astype(np.float64)

    pk1 = np.zeros((64, PK1_COLS))
    pk1[0:D, 0:64] = W_score
    pk1[0:8, 64:192] = E2q
    pk2 = np.zeros((128, 41))
    pk2[:, 0:8] = O1
    pk2[:, 8:40] = O2
    pk2[:, 40] = b_score2

    return dict(
        trig=trig.astype(BF16),
        pk1=pk1.astype(BF16),
        pk2=pk2.astype(BF16),
        pk3=G.astype(BF16),
        ones1=np.ones((1, S), BF16),
    ), gsh, M_alpha, c_alpha


def _numpy_fallback(inputs):
    """Exact collapsed computation in numpy (general freq rows)."""
    x = np.asarray(inputs["x"], np.float64)
    f = (np.asarray(inputs["freq_matrix"], np.float64)
         * np.asarray(inputs["freq_scale"], np.float64))
    Wq = np.asarray(inputs["Wq"], np.float64); bq = np.asarray(inputs["bq"], np.float64)
    Wk1 = np.asarray(inputs["Wk1"], np.float64); bk1 = np.asarray(inputs["bk1"], np.float64)
    Wqi = np.asarray(inputs["Wqi"], np.float64); bqi = np.asarray(inputs["bqi"], np.float64)
    Wki = np.asarray(inputs["Wki"], np.float64); bki = np.asarray(inputs["bki"], np.float64)
    Wg = np.asarray(inputs["Wg"], np.float64); bg = np.asarray(inputs["bg"], np.float64)
    Wp = np.asarray(inputs["Wp"], np.float64); bp = np.asarray(inputs["bp"], np.float64)
    ph = np.asarray(inputs["phase"], np.float64)

    u = Wki @ Wk1[:, 0]
    v = Wki @ bk1 + bki
    q = (x @ Wq.T + bq) @ Wqi.T + bqi
    qh = q.reshape(B, S, H, HD)
    alpha = np.einsum("bshe,he->bsh", qh, u.reshape(H, HD)) / np.sqrt(HD)
    beta = np.einsum("bshe,he->bsh", qh, v.reshape(H, HD)) / np.sqrt(HD)
    sc = alpha[..., None, :, None] * f[None, None, :, None, :] \
        + beta[..., None, :, None]
    sc -= sc.max(-1, keepdims=True)
    e = np.exp(sc)
    attn = e / e.sum(-1, keepdims=True)
    aw = attn.mean(-2)
    t = np.linspace(0.0, 1.0, S)
    sig = TWO_PI * t[None, :, None, None] * f[None, None] + ph[None, None]
    ffs = np.sin(sig) * aw
    ffc = np.cos(sig) * aw
    ff = np.concatenate([ffs, ffc], axis=-1).reshape(B, S, D * 2 * F)
    ci = np.concatenate([x, ff], axis=-1)
    gate = 1.0 / (1.0 + np.exp(-(ci @ Wg.T + bg)))
    pp = ci @ Wp.T + bp
    silu = pp / (1.0 + np.exp(-pp))
    return (x + gate * silu).astype(np.float32)


def kernel(**inputs) -> np.ndarray:
    x = np.asarray(inputs["x"], np.float32)

    f = (np.asarray(inputs["freq_matrix"], np.float64)
         * np.asarray(inputs["freq_scale"], np.float64))
    if not np.all(f == f[0:1]):
        return _numpy_fallback(inputs)

    params, gsh, M_alpha, c_alpha = _fold_params(inputs)

    # exp-overflow guard (score = alpha*(g-gc); needs |score| < ~85)
    xmaxn = np.linalg.norm(x.reshape(-1, D), axis=1).max()
    amax = np.linalg.norm(M_alpha, axis=1).max() * xmaxn + np.abs(c_alpha).max()
    if amax * np.abs(gsh).max() > 85.0:
        return _numpy_fallback(inputs)

    use_bias = bool(np.abs(np.asarray(params["pk2"], np.float64)[:, 40]).max() > 0)
    key = f"prog{int(use_bias)}"
    if key not in _CACHE:
        _CACHE[key] = _build_program(use_bias=use_bias)
    nc = _CACHE[key]

    from concourse.bass_utils import run_bass_kernel_spmd

    in_maps = _make_inmaps(x, params)
    res = run_bass_kernel_spmd(nc, in_maps, core_ids=list(range(N_CORES)))
    return _finish(x, res)


if __name__ == "__main__":
    import reference
    ins = {k: np.asarray(v) for k, v in reference.setup_inputs().items()}
    got = kernel(**ins)
    import jax.numpy as jnp
    exp = np.asarray(reference.reference(**{k: jnp.asarray(v) for k, v in ins.items()}))
    err = np.linalg.norm(got - exp) / np.linalg.norm(exp)
    print("rel err:", err)
